# revision 31
# baseline (speedup 1.0000x reference)
"""Trainium2 Bass kernel for nn_ColorFeatureExtractor (per-image KMeans color
extraction). Pure data parallel: image b -> core b. Each core runs 100 Lloyd
iterations entirely on-chip and streams back per-iteration counts + centers
trajectories; the host selects the convergence iteration (faithful to the
reference's global-allclose freeze semantics) and assembles the [B,K,K,4]
output."""
import sys
import numpy as np

for _p in ("/opt/trn_rl_repo", "/root/.axon_site/_ro/trn_rl_repo"):
    if _p not in sys.path:
        sys.path.append(_p)

K = 5
N = 224 * 224          # pixels per image
P = 128                # partitions
F = N // P             # 392 free elems per partition
import os
ITERS = int(os.environ.get("KM_ITERS", "100"))
RTOL, ATOL = 1e-5, 1e-8
OUT_LEN = 500 + 101 * 15   # counts traj + centers traj

_CACHE = {}


def _build_nc():
    import concourse.bass as bass
    import concourse.mybir as mybir
    from concourse import bacc, tile

    f32 = mybir.dt.float32
    Alu = mybir.AluOpType
    Act = mybir.ActivationFunctionType

    nc = bacc.Bacc(None, target_bir_lowering=False)
    xp = nc.dram_tensor("xp", [3, N], f32, kind="ExternalInput")
    cbin = nc.dram_tensor("cbin", [1, 20], f32, kind="ExternalInput")
    outv = nc.dram_tensor("outv", [1, OUT_LEN], f32, kind="ExternalOutput")

    with tile.TileContext(nc) as tc:
        with (
            tc.tile_pool(name="persist", bufs=1) as pp,
            tc.tile_pool(name="sbig", bufs=2) as sb,
            tc.tile_pool(name="scr", bufs=3) as scr,
            tc.tile_pool(name="small", bufs=2) as sm,
            tc.tile_pool(name="psum", bufs=2, space=bass.MemorySpace.PSUM) as ps,
        ):
            # ---- persistent tiles ----
            px = pp.tile([P, F], f32, tag="px")
            py = pp.tile([P, F], f32, tag="py")
            pz = pp.tile([P, F], f32, tag="pz")
            ones_col = pp.tile([P, 1], f32, tag="ones_col")    # matmul lhsT for col-sum
            ones_row = pp.tile([1, P], f32, tag="ones_row")    # matmul lhsT for broadcast
            tot3 = pp.tile([1, 3], f32, tag="tot3")            # sum of px/py/pz
            counts_st = pp.tile([1, 500], f32, tag="counts_st")
            cent_st = pp.tile([1, 101 * 15], f32, tag="cent_st")
            # constants for gpsimd tail ops (gpsimd has no tensor_scalar)
            nconst = pp.tile([1, 1], f32, tag="nconst")
            halfc = pp.tile([1, 1], f32, tag="halfc")
            twoc = pp.tile([1, 1], f32, tag="twoc")

            nc.vector.memset(counts_st[:], 0.0)
            nc.vector.memset(cent_st[:], 0.0)
            nc.vector.memset(nconst[:], float(N))
            nc.vector.memset(halfc[:], -0.5)
            nc.vector.memset(twoc[:], 2.0)
            xap = xp[:].rearrange("c (p f) -> c p f", p=P)
            nc.sync.dma_start(out=px[:], in_=xap[0])
            nc.sync.dma_start(out=py[:], in_=xap[1])
            nc.sync.dma_start(out=pz[:], in_=xap[2])
            cb0 = pp.tile([1, 20], f32, tag="cb0")
            nc.sync.dma_start(out=cb0[:], in_=cbin[:])

            nc.vector.memset(ones_col[:], 1.0)
            nc.vector.memset(ones_row[:], 1.0)

            # pixels = x + 1e-8, vector-owned; gpsimd gets private copies so
            # its loop-body ops never need cross-engine waits (HW structs have
            # very few sync-wait slots)
            nc.vector.tensor_scalar(px[:], px[:], 1e-8, None, Alu.add)
            nc.vector.tensor_scalar(py[:], py[:], 1e-8, None, Alu.add)
            nc.vector.tensor_scalar(pz[:], pz[:], 1e-8, None, Alu.add)


            planes0 = (px, py, pz)
            # totals: [1,3] = sum of each plane
            totc = pp.tile([P, 3], f32, tag="totc")
            nc.vector.tensor_reduce(totc[:, 0:1], px[:], mybir.AxisListType.X, Alu.add)
            nc.vector.tensor_reduce(totc[:, 1:2], py[:], mybir.AxisListType.X, Alu.add)
            nc.vector.tensor_reduce(totc[:, 2:3], pz[:], mybir.AxisListType.X, Alu.add)
            tot3_ps = ps.tile([1, 3], f32, tag="tot3ps")
            nc.tensor.matmul(tot3_ps[:], ones_col[:], totc[:], start=True, stop=True)
            nc.vector.tensor_copy(tot3[:], tot3_ps[:])

            # interleaved pixel tile [p, f*3] = (x,y,z) per pixel, for the
            # one-TT-per-cluster product in phase 3
            pint = pp.tile([P, 3 * F], f32, tag="pint")
            for d in range(3):
                nc.vector.tensor_copy(
                    pint[:].rearrange("p (f d) -> p d f", d=3)[:, d], planes0[d][:]
                )

            # initial centers into trajectory + initial rep broadcast
            nc.scalar.copy(cent_st[0:1, 0:15], cb0[0:1, 0:15])

            cb0v = pp.tile([1, 20], f32, tag="cb0v")
            nc.vector.tensor_copy(cb0v[:], cb0[:])
            rep_ps0 = ps.tile([P, 20], f32, tag="repps")
            nc.tensor.matmul(rep_ps0[:], ones_row[:], cb0v[:], start=True, stop=True)
            rep = sb.tile([P, 20], f32, tag="rep")
            nc.vector.tensor_copy(rep[:], rep_ps0[:])

            for t in range(1, ITERS + 1):
                # ---------- phase 1: scores s_k = px*cx + py*cy + pz*cz + b ----------
                s5 = sb.tile([P, 5 * F], f32, tag="s5")
                u_tiles = []
                for k in range(5):
                    u = scr.tile([P, F], f32, tag=f"u{k}")
                    # u = px*cx_k + b_k (ACT free affine with AP scale/bias)
                    nc.scalar.activation(
                        u[:], px[:], Act.Identity,
                        bias=rep[:, 15 + k : 16 + k], scale=rep[:, 3 * k : 3 * k + 1],
                    )
                    u_tiles.append(u)
                for k in range(5):
                    v = scr.tile([P, F], f32, tag=f"v{k}")
                    nc.vector.scalar_tensor_tensor(
                        v[:], py[:], rep[:, 3 * k + 1 : 3 * k + 2], u_tiles[k][:],
                        Alu.mult, Alu.add,
                    )
                    nc.vector.scalar_tensor_tensor(
                        s5[:, k * F : (k + 1) * F], pz[:],
                        rep[:, 3 * k + 2 : 3 * k + 3], v[:], Alu.mult, Alu.add,
                    )

                # ---------- phase 2: m = max_k s_k (one segmented reduce) ----------
                m = sb.tile([P, F], f32, tag="m")
                nc.vector.tensor_reduce(
                    m[:], s5[:].rearrange("p (k f) -> p f k", k=5),
                    mybir.AxisListType.X, Alu.max,
                )

                # ---------- phase 3: masks, counts, products, sums ----------
                # acc_d: 0:4 cnt0..3 | 4:16 S k-major | 16 csum
                acc_d = sb.tile([P, 17], f32, tag="acc_d")
                junk_a = scr.tile([P, F], f32, tag="junk_a")
                maskQ = sb.tile([P, 4 * F], f32, tag="maskQ")
                mask_tiles = [maskQ[:, k * F : (k + 1) * F] for k in range(4)]
                # GPS's product clusters (1,2,3) first so gpsimd starts early
                for k in (1, 2, 3, 0):
                    nc.vector.tensor_tensor(
                        mask_tiles[k], s5[:, k * F : (k + 1) * F], m[:], Alu.is_equal
                    )
                for k in range(4):
                    nc.scalar.activation(
                        junk_a[:], mask_tiles[k], Act.Identity,
                        accum_out=acc_d[:, k : k + 1],
                    )
                junk4 = sm.tile([P, 4], f32, tag="junk4")
                nc.scalar.activation(
                    junk4[:], acc_d[:, 0:4], Act.Identity,
                    accum_out=acc_d[:, 16:17],
                )

                # products: clusters 0 on DVE (strided one-op); 1,2,3 on GpSimd
                # (plain contiguous per-channel TTs). prod memory is d-major.
                prods = []
                for k in range(4):
                    prod3 = scr.tile([P, 3 * F], f32, tag=f"prod{k}")
                    prods.append(prod3)
                nc.vector.tensor_tensor(
                    prods[0][:].rearrange("p (d f) -> p f d", f=F),
                    mask_tiles[0].rearrange("p (f o) -> p f o", o=1).broadcast_to((P, F, 3)),
                    pint[:].rearrange("p (f d) -> p f d", d=3),
                    Alu.mult,
                )
                for k in (1, 2, 3):
                    nc.gpsimd.tensor_tensor(
                        prods[k][:].rearrange("p (d f) -> p f d", f=F),
                        mask_tiles[k].rearrange("p (f o) -> p f o", o=1).broadcast_to((P, F, 3)),
                        pint[:].rearrange("p (f d) -> p f d", d=3),
                        Alu.mult,
                    )
                # sums: clusters 0,1 via DVE segmented reduce; 2,3 via ACT accum
                for k in (0, 1):
                    nc.vector.tensor_reduce(
                        acc_d[:, 4 + 3 * k : 7 + 3 * k],
                        prods[k][:].rearrange("p (d f) -> p d f", d=3),
                        mybir.AxisListType.X, Alu.add,
                    )
                for k in (2, 3):
                    for d in range(3):
                        nc.scalar.activation(
                            junk_a[:], prods[k][:, d * F : (d + 1) * F], Act.Identity,
                            accum_out=acc_d[:, 4 + 3 * k + d : 5 + 3 * k + d],
                        )

                # ---------- tail: totals -> new centers ----------
                tot = ps.tile([1, 17], f32, tag="tot")
                nc.tensor.matmul(tot[:], ones_col[:], acc_d[:], start=True, stop=True)
                tots = sm.tile([1, 17], f32, tag="tots")
                nc.scalar.copy(tots[:], tot[:])

                cnts = sm.tile([1, 5], f32, tag="cnts")
                nc.gpsimd.tensor_copy(cnts[0:1, 0:4], tots[0:1, 0:4])
                nc.gpsimd.tensor_tensor(
                    cnts[0:1, 4:5], nconst[:], tots[0:1, 16:17], Alu.subtract
                )

                S15 = sm.tile([1, 15], f32, tag="S15")
                s4p = sm.tile([1, 6], f32, tag="s4p")
                nc.gpsimd.tensor_copy(S15[0:1, 0:12], tots[0:1, 4:16])
                # sum over k of S_kd via gpsimd add tree (keeps tail on one engine)
                nc.gpsimd.tensor_tensor(
                    s4p[0:1, 0:3], tots[0:1, 4:7], tots[0:1, 7:10], Alu.add
                )
                nc.gpsimd.tensor_tensor(
                    s4p[0:1, 3:6], tots[0:1, 10:13], tots[0:1, 13:16], Alu.add
                )
                nc.gpsimd.tensor_tensor(
                    s4p[0:1, 0:3], s4p[0:1, 0:3], s4p[0:1, 3:6], Alu.add
                )
                nc.gpsimd.tensor_tensor(S15[0:1, 12:15], tot3[:], s4p[0:1, 0:3], Alu.subtract)

                recip = sm.tile([1, 5], f32, tag="recip")
                nc.vector.reciprocal(recip[:], cnts[:])

                cb = sm.tile([1, 20], f32, tag="cb")
                nc.gpsimd.tensor_tensor(
                    cb[0:1, 0:15].rearrange("p (k d) -> p k d", d=3),
                    S15[:].rearrange("p (k d) -> p k d", d=3),
                    recip[:].rearrange("p (k o) -> p k o", o=1).broadcast_to((1, 5, 3)),
                    Alu.mult,
                )

                sq = sm.tile([1, 15], f32, tag="sq")
                c2 = sm.tile([1, 5], f32, tag="c2")
                nc.gpsimd.tensor_tensor(sq[:], cb[0:1, 0:15], cb[0:1, 0:15], Alu.mult)
                nc.gpsimd.tensor_tensor(
                    c2[:], sq[0:1, 0:15:3], sq[0:1, 1:15:3], Alu.add
                )
                nc.gpsimd.tensor_tensor(c2[:], c2[:], sq[0:1, 2:15:3], Alu.add)
                nc.gpsimd.tensor_tensor(
                    c2[:], c2[:], halfc[:].broadcast_to((1, 5)), Alu.mult
                )
                nc.gpsimd.tensor_tensor(
                    cb[0:1, 15:20], twoc[:].broadcast_to((1, 5)), c2[:], Alu.add
                )

                # store trajectories (ScalarE, off critical path)
                nc.scalar.copy(counts_st[0:1, 5 * (t - 1) : 5 * t], cnts[:])
                nc.scalar.copy(cent_st[0:1, 15 * t : 15 * (t + 1)], cb[0:1, 0:15])

                # broadcast for next iteration (gpsimd partition broadcast)
                rep = sb.tile([P, 20], f32, tag="rep")
                nc.gpsimd.partition_broadcast(rep[:], cb[0:1, :], channels=P)

            nc.sync.dma_start(out=outv[0:1, 0:500], in_=counts_st[:])
            nc.sync.dma_start(out=outv[0:1, 500:OUT_LEN], in_=cent_st[:])
    nc.compile()
    return nc


def _get_nc():
    if "nc" not in _CACHE:
        _CACHE["nc"] = _build_nc()
    return _CACHE["nc"]


def _host_finalize(counts_all, cent_all):
    """counts_all [B,100,5], cent_all [B,101,15] -> [B,K,K,4] per reference."""
    B = counts_all.shape[0]
    prev = cent_all[:, :-1, :]   # centers entering iter t (t=1..100)
    new = cent_all[:, 1:, :]     # new_centers at iter t
    with np.errstate(invalid="ignore"):
        ok = np.abs(prev - new) <= np.float32(ATOL) + np.float32(RTOL) * np.abs(new)
    conv_t = np.all(ok, axis=(0, 2))          # [100] global allclose per iter
    idx = np.nonzero(conv_t)[0]
    T = int(idx[0]) + 1 if len(idx) else ITERS + 1
    L = min(T, ITERS)
    centers = cent_all[:, T - 1].reshape(B, K, 3)
    percentages = counts_all[:, L - 1] / np.float32(N)
    centers = np.clip(centers, 0.0, 1.0)
    percentages = np.clip(percentages, 0.0, 1.0)
    color_info = np.concatenate([centers, percentages[..., None]], axis=2).astype(np.float32)
    color_info = np.nan_to_num(color_info, nan=0.0, posinf=1.0, neginf=0.0)
    sort_idx = np.argsort(-color_info[:, :, 3], axis=1, kind="stable")
    return color_info[sort_idx]


def _make_inputs(x, init_idx):
    B = x.shape[0]
    x = np.ascontiguousarray(np.asarray(x, dtype=np.float32))
    init_idx = np.asarray(init_idx).astype(np.int64)
    hh, ww = init_idx // 224, init_idx % 224
    in_maps = []
    for b in range(B):
        c0 = (x[b, :, hh, ww] + np.float32(1e-8)).astype(np.float32)  # [5,3]
        cb0 = np.zeros((1, 20), np.float32)
        cb0[0, :15] = c0.reshape(15)
        c2 = (c0 * c0).sum(axis=1, dtype=np.float32)
        cb0[0, 15:20] = np.float32(2.0) - np.float32(0.5) * c2
        in_maps.append({"xp": x[b].reshape(3, N), "cbin": cb0})
    return in_maps


def kernel(x, init_idx):
    from concourse.bass_utils import run_bass_kernel_spmd

    nc = _get_nc()
    in_maps = _make_inputs(x, init_idx)
    res = run_bass_kernel_spmd(nc, in_maps, list(range(8)))
    outs = [np.asarray(r["outv"]).reshape(OUT_LEN) for r in res.results]
    counts_all = np.stack([o[0:500].reshape(100, 5) for o in outs])
    cent_all = np.stack([o[500:OUT_LEN].reshape(101, 15) for o in outs])
    return _host_finalize(counts_all, cent_all)


# revision 35
# speedup vs baseline: 1.3273x; 1.3273x over previous
"""Trainium2 Bass kernel for nn_ColorFeatureExtractor (per-image KMeans color
extraction). Pure data parallel: image b -> core b. Each core runs 100 Lloyd
iterations entirely on-chip and streams back per-iteration counts + centers
trajectories; the host selects the convergence iteration (faithful to the
reference's global-allclose freeze semantics) and assembles the [B,K,K,4]
output."""
import sys
import numpy as np

for _p in ("/opt/trn_rl_repo", "/root/.axon_site/_ro/trn_rl_repo"):
    if _p not in sys.path:
        sys.path.append(_p)

K = 5
N = 224 * 224          # pixels per image
P = 128                # partitions
F = N // P             # 392 free elems per partition
import os
ITERS = int(os.environ.get("KM_ITERS", "100"))
RTOL, ATOL = 1e-5, 1e-8
OUT_LEN = 500 + 101 * 15   # counts traj + centers traj

_CACHE = {}


def _build_nc():
    import concourse.bass as bass
    import concourse.mybir as mybir
    from concourse import bacc, tile

    f32 = mybir.dt.float32
    Alu = mybir.AluOpType
    Act = mybir.ActivationFunctionType

    nc = bacc.Bacc(None, target_bir_lowering=False)
    xp = nc.dram_tensor("xp", [3, N], f32, kind="ExternalInput")
    cbin = nc.dram_tensor("cbin", [1, 20], f32, kind="ExternalInput")
    outv = nc.dram_tensor("outv", [1, OUT_LEN], f32, kind="ExternalOutput")

    with tile.TileContext(nc) as tc:
        with (
            tc.tile_pool(name="persist", bufs=1) as pp,
            tc.tile_pool(name="sbig", bufs=2) as sb,
            tc.tile_pool(name="scr", bufs=3) as scr,
            tc.tile_pool(name="small", bufs=2) as sm,
            tc.tile_pool(name="psum", bufs=2, space=bass.MemorySpace.PSUM) as ps,
        ):
            # ---- persistent tiles ----
            px = pp.tile([P, F], f32, tag="px")
            py = pp.tile([P, F], f32, tag="py")
            pz = pp.tile([P, F], f32, tag="pz")
            ones_col = pp.tile([P, 1], f32, tag="ones_col")    # matmul lhsT for col-sum
            ones_row = pp.tile([1, P], f32, tag="ones_row")    # matmul lhsT for broadcast
            tot3 = pp.tile([1, 3], f32, tag="tot3")            # sum of px/py/pz
            counts_st = pp.tile([1, 500], f32, tag="counts_st")
            cent_st = pp.tile([1, 101 * 15], f32, tag="cent_st")
            # constants for gpsimd tail ops (gpsimd has no tensor_scalar)
            nconst = pp.tile([1, 1], f32, tag="nconst")
            halfc = pp.tile([1, 1], f32, tag="halfc")
            twoc = pp.tile([1, 1], f32, tag="twoc")

            nc.vector.memset(counts_st[:], 0.0)
            nc.vector.memset(cent_st[:], 0.0)
            nc.vector.memset(nconst[:], float(N))
            nc.vector.memset(halfc[:], -0.5)
            nc.vector.memset(twoc[:], 2.0)
            xap = xp[:].rearrange("c (p f) -> c p f", p=P)
            nc.sync.dma_start(out=px[:], in_=xap[0])
            nc.sync.dma_start(out=py[:], in_=xap[1])
            nc.sync.dma_start(out=pz[:], in_=xap[2])
            cb0 = pp.tile([1, 20], f32, tag="cb0")
            nc.sync.dma_start(out=cb0[:], in_=cbin[:])

            nc.vector.memset(ones_col[:], 1.0)
            nc.vector.memset(ones_row[:], 1.0)

            # pixels = x + 1e-8, vector-owned; gpsimd gets private copies so
            # its loop-body ops never need cross-engine waits (HW structs have
            # very few sync-wait slots)
            nc.vector.tensor_scalar(px[:], px[:], 1e-8, None, Alu.add)
            nc.vector.tensor_scalar(py[:], py[:], 1e-8, None, Alu.add)
            nc.vector.tensor_scalar(pz[:], pz[:], 1e-8, None, Alu.add)


            planes0 = (px, py, pz)
            # totals: [1,3] = sum of each plane
            totc = pp.tile([P, 3], f32, tag="totc")
            nc.vector.tensor_reduce(totc[:, 0:1], px[:], mybir.AxisListType.X, Alu.add)
            nc.vector.tensor_reduce(totc[:, 1:2], py[:], mybir.AxisListType.X, Alu.add)
            nc.vector.tensor_reduce(totc[:, 2:3], pz[:], mybir.AxisListType.X, Alu.add)
            tot3_ps = ps.tile([1, 3], f32, tag="tot3ps")
            nc.tensor.matmul(tot3_ps[:], ones_col[:], totc[:], start=True, stop=True)
            nc.vector.tensor_copy(tot3[:], tot3_ps[:])

            # interleaved pixel tile [p, f*3] = (x,y,z) per pixel, for the
            # one-TT-per-cluster product in phase 3
            pint = pp.tile([P, 3 * F], f32, tag="pint")
            for d in range(3):
                nc.vector.tensor_copy(
                    pint[:].rearrange("p (f d) -> p d f", d=3)[:, d], planes0[d][:]
                )

            # initial centers into trajectory + initial rep broadcast
            nc.scalar.copy(cent_st[0:1, 0:15], cb0[0:1, 0:15])

            cb0v = pp.tile([1, 20], f32, tag="cb0v")
            nc.vector.tensor_copy(cb0v[:], cb0[:])
            rep_ps0 = ps.tile([P, 20], f32, tag="repps")
            nc.tensor.matmul(rep_ps0[:], ones_row[:], cb0v[:], start=True, stop=True)
            rep = sb.tile([P, 20], f32, tag="rep")
            nc.vector.tensor_copy(rep[:], rep_ps0[:])

            for t in range(1, ITERS + 1):
                # ---------- phase 1: scores s_k = px*cx + py*cy + pz*cz + b ----------
                s5 = sb.tile([P, 5 * F], f32, tag="s5")
                u_tiles = []
                for k in range(5):
                    u = scr.tile([P, F], f32, tag=f"u{k}")
                    # u = px*cx_k + b_k (ACT free affine with AP scale/bias)
                    nc.scalar.activation(
                        u[:], px[:], Act.Identity,
                        bias=rep[:, 15 + k : 16 + k], scale=rep[:, 3 * k : 3 * k + 1],
                    )
                    u_tiles.append(u)
                for k in range(5):
                    v = scr.tile([P, F], f32, tag=f"v{k}")
                    nc.vector.scalar_tensor_tensor(
                        v[:], py[:], rep[:, 3 * k + 1 : 3 * k + 2], u_tiles[k][:],
                        Alu.mult, Alu.add,
                    )
                    nc.vector.scalar_tensor_tensor(
                        s5[:, k * F : (k + 1) * F], pz[:],
                        rep[:, 3 * k + 2 : 3 * k + 3], v[:], Alu.mult, Alu.add,
                    )

                # ---------- phase 2: m = max_k s_k (one segmented reduce) ----------
                m = sb.tile([P, F], f32, tag="m")
                nc.vector.tensor_reduce(
                    m[:], s5[:].rearrange("p (k f) -> p f k", k=5),
                    mybir.AxisListType.X, Alu.max,
                )

                # ---------- phase 3: masks (one fused TT), counts, products, sums ----------
                # acc_d: 0:4 cnt0..3 | 4:16 S k-major | 16 csum
                acc_d = sb.tile([P, 17], f32, tag="acc_d")
                junk_a = scr.tile([P, F], f32, tag="junk_a")
                maskQ = sb.tile([P, 4 * F], f32, tag="maskQ")
                nc.vector.tensor_tensor(
                    maskQ[:].rearrange("p (k f) -> p k f", k=4),
                    s5[:, 0 : 4 * F].rearrange("p (k f) -> p k f", k=4),
                    m[:].rearrange("p (o f) -> p o f", o=1).broadcast_to((P, 4, F)),
                    Alu.is_equal,
                )
                mask_tiles = [maskQ[:, k * F : (k + 1) * F] for k in range(4)]
                for k in range(4):
                    nc.scalar.activation(
                        junk_a[:], mask_tiles[k], Act.Identity,
                        accum_out=acc_d[:, k : k + 1],
                    )
                junk4 = sm.tile([P, 4], f32, tag="junk4")
                nc.scalar.activation(
                    junk4[:], acc_d[:, 0:4], Act.Identity,
                    accum_out=acc_d[:, 16:17],
                )

                # products: clusters 0 on DVE (strided one-op); 1,2,3 on GpSimd
                # (plain contiguous per-channel TTs). prod memory is d-major.
                prods = []
                for k in range(4):
                    prod3 = scr.tile([P, 3 * F], f32, tag=f"prod{k}")
                    prods.append(prod3)
                nc.vector.tensor_tensor(
                    prods[0][:].rearrange("p (d f) -> p f d", f=F),
                    mask_tiles[0].rearrange("p (f o) -> p f o", o=1).broadcast_to((P, F, 3)),
                    pint[:].rearrange("p (f d) -> p f d", d=3),
                    Alu.mult,
                )
                for k in (1, 2, 3):
                    nc.gpsimd.tensor_tensor(
                        prods[k][:].rearrange("p (d f) -> p f d", f=F),
                        mask_tiles[k].rearrange("p (f o) -> p f o", o=1).broadcast_to((P, F, 3)),
                        pint[:].rearrange("p (f d) -> p f d", d=3),
                        Alu.mult,
                    )
                # sums: clusters 0,1 via DVE segmented reduce; 2,3 via ACT accum
                for k in (0, 1):
                    nc.vector.tensor_reduce(
                        acc_d[:, 4 + 3 * k : 7 + 3 * k],
                        prods[k][:].rearrange("p (d f) -> p d f", d=3),
                        mybir.AxisListType.X, Alu.add,
                    )
                for k in (2, 3):
                    for d in range(3):
                        nc.scalar.activation(
                            junk_a[:], prods[k][:, d * F : (d + 1) * F], Act.Identity,
                            accum_out=acc_d[:, 4 + 3 * k + d : 5 + 3 * k + d],
                        )

                # ---------- tail: totals -> new centers ----------
                tot = ps.tile([1, 17], f32, tag="tot")
                nc.tensor.matmul(tot[:], ones_col[:], acc_d[:], start=True, stop=True)
                tots = sm.tile([1, 17], f32, tag="tots")
                nc.scalar.copy(tots[:], tot[:])

                cnts = sm.tile([1, 5], f32, tag="cnts")
                nc.gpsimd.tensor_copy(cnts[0:1, 0:4], tots[0:1, 0:4])
                nc.gpsimd.tensor_tensor(
                    cnts[0:1, 4:5], nconst[:], tots[0:1, 16:17], Alu.subtract
                )

                S15 = sm.tile([1, 15], f32, tag="S15")
                s4p = sm.tile([1, 3], f32, tag="s4p")
                nc.gpsimd.tensor_copy(S15[0:1, 0:12], tots[0:1, 4:16])
                # sum over k of S_kd: view cols 4..16 as [d, k(stride3)], reduce X
                nc.vector.tensor_reduce(
                    s4p[:], tots[0:1, 4:16].rearrange("p (k d) -> p d k", d=3),
                    mybir.AxisListType.X, Alu.add,
                )
                nc.gpsimd.tensor_tensor(S15[0:1, 12:15], tot3[:], s4p[:], Alu.subtract)

                recip = sm.tile([1, 5], f32, tag="recip")
                nc.vector.reciprocal(recip[:], cnts[:])

                cb = sm.tile([1, 20], f32, tag="cb")
                nc.gpsimd.tensor_tensor(
                    cb[0:1, 0:15].rearrange("p (k d) -> p k d", d=3),
                    S15[:].rearrange("p (k d) -> p k d", d=3),
                    recip[:].rearrange("p (k o) -> p k o", o=1).broadcast_to((1, 5, 3)),
                    Alu.mult,
                )

                sq = sm.tile([1, 15], f32, tag="sq")
                c2 = sm.tile([1, 5], f32, tag="c2")
                nc.gpsimd.tensor_tensor(sq[:], cb[0:1, 0:15], cb[0:1, 0:15], Alu.mult)
                nc.vector.tensor_reduce(
                    c2[:], sq[:].rearrange("p (k d) -> p k d", d=3),
                    mybir.AxisListType.X, Alu.add,
                )
                nc.gpsimd.tensor_tensor(
                    c2[:], c2[:], halfc[:].broadcast_to((1, 5)), Alu.mult
                )
                nc.gpsimd.tensor_tensor(
                    cb[0:1, 15:20], twoc[:].broadcast_to((1, 5)), c2[:], Alu.add
                )

                # store trajectories (ScalarE, off critical path)
                nc.scalar.copy(counts_st[0:1, 5 * (t - 1) : 5 * t], cnts[:])
                nc.scalar.copy(cent_st[0:1, 15 * t : 15 * (t + 1)], cb[0:1, 0:15])

                # broadcast for next iteration
                rep_ps = ps.tile([P, 20], f32, tag="repps")
                nc.tensor.matmul(rep_ps[:], ones_row[:], cb[:], start=True, stop=True)
                rep = sb.tile([P, 20], f32, tag="rep")
                nc.scalar.copy(rep[:], rep_ps[:])

            nc.sync.dma_start(out=outv[0:1, 0:500], in_=counts_st[:])
            nc.sync.dma_start(out=outv[0:1, 500:OUT_LEN], in_=cent_st[:])
    nc.compile()
    return nc


def _get_nc():
    if "nc" not in _CACHE:
        _CACHE["nc"] = _build_nc()
    return _CACHE["nc"]


def _host_finalize(counts_all, cent_all):
    """counts_all [B,100,5], cent_all [B,101,15] -> [B,K,K,4] per reference."""
    B = counts_all.shape[0]
    prev = cent_all[:, :-1, :]   # centers entering iter t (t=1..100)
    new = cent_all[:, 1:, :]     # new_centers at iter t
    with np.errstate(invalid="ignore"):
        ok = np.abs(prev - new) <= np.float32(ATOL) + np.float32(RTOL) * np.abs(new)
    conv_t = np.all(ok, axis=(0, 2))          # [100] global allclose per iter
    idx = np.nonzero(conv_t)[0]
    T = int(idx[0]) + 1 if len(idx) else ITERS + 1
    L = min(T, ITERS)
    centers = cent_all[:, T - 1].reshape(B, K, 3)
    percentages = counts_all[:, L - 1] / np.float32(N)
    centers = np.clip(centers, 0.0, 1.0)
    percentages = np.clip(percentages, 0.0, 1.0)
    color_info = np.concatenate([centers, percentages[..., None]], axis=2).astype(np.float32)
    color_info = np.nan_to_num(color_info, nan=0.0, posinf=1.0, neginf=0.0)
    sort_idx = np.argsort(-color_info[:, :, 3], axis=1, kind="stable")
    return color_info[sort_idx]


def _make_inputs(x, init_idx):
    B = x.shape[0]
    x = np.ascontiguousarray(np.asarray(x, dtype=np.float32))
    init_idx = np.asarray(init_idx).astype(np.int64)
    hh, ww = init_idx // 224, init_idx % 224
    in_maps = []
    for b in range(B):
        c0 = (x[b, :, hh, ww] + np.float32(1e-8)).astype(np.float32)  # [5,3]
        cb0 = np.zeros((1, 20), np.float32)
        cb0[0, :15] = c0.reshape(15)
        c2 = (c0 * c0).sum(axis=1, dtype=np.float32)
        cb0[0, 15:20] = np.float32(2.0) - np.float32(0.5) * c2
        in_maps.append({"xp": x[b].reshape(3, N), "cbin": cb0})
    return in_maps


def kernel(x, init_idx):
    from concourse.bass_utils import run_bass_kernel_spmd

    nc = _get_nc()
    in_maps = _make_inputs(x, init_idx)
    res = run_bass_kernel_spmd(nc, in_maps, list(range(8)))
    outs = [np.asarray(r["outv"]).reshape(OUT_LEN) for r in res.results]
    counts_all = np.stack([o[0:500].reshape(100, 5) for o in outs])
    cent_all = np.stack([o[500:OUT_LEN].reshape(101, 15) for o in outs])
    return _host_finalize(counts_all, cent_all)


# revision 36
# speedup vs baseline: 1.3608x; 1.0252x over previous
"""Trainium2 Bass kernel for nn_ColorFeatureExtractor (per-image KMeans color
extraction). Pure data parallel: image b -> core b. Each core runs 100 Lloyd
iterations entirely on-chip and streams back per-iteration counts + centers
trajectories; the host selects the convergence iteration (faithful to the
reference's global-allclose freeze semantics) and assembles the [B,K,K,4]
output."""
import sys
import numpy as np

for _p in ("/opt/trn_rl_repo", "/root/.axon_site/_ro/trn_rl_repo"):
    if _p not in sys.path:
        sys.path.append(_p)

K = 5
N = 224 * 224          # pixels per image
P = 128                # partitions
F = N // P             # 392 free elems per partition
import os
ITERS = int(os.environ.get("KM_ITERS", "100"))
RTOL, ATOL = 1e-5, 1e-8
OUT_LEN = 500 + 101 * 15   # counts traj + centers traj

_CACHE = {}


def _build_nc():
    import concourse.bass as bass
    import concourse.mybir as mybir
    from concourse import bacc, tile

    f32 = mybir.dt.float32
    Alu = mybir.AluOpType
    Act = mybir.ActivationFunctionType

    nc = bacc.Bacc(None, target_bir_lowering=False)
    xp = nc.dram_tensor("xp", [3, N], f32, kind="ExternalInput")
    cbin = nc.dram_tensor("cbin", [1, 20], f32, kind="ExternalInput")
    outv = nc.dram_tensor("outv", [1, OUT_LEN], f32, kind="ExternalOutput")

    with tile.TileContext(nc) as tc:
        with (
            tc.tile_pool(name="persist", bufs=1) as pp,
            tc.tile_pool(name="sbig", bufs=2) as sb,
            tc.tile_pool(name="scr", bufs=3) as scr,
            tc.tile_pool(name="small", bufs=2) as sm,
            tc.tile_pool(name="psum", bufs=2, space=bass.MemorySpace.PSUM) as ps,
        ):
            # ---- persistent tiles ----
            px = pp.tile([P, F], f32, tag="px")
            py = pp.tile([P, F], f32, tag="py")
            pz = pp.tile([P, F], f32, tag="pz")
            ones_col = pp.tile([P, 1], f32, tag="ones_col")    # matmul lhsT for col-sum
            ones_row = pp.tile([1, P], f32, tag="ones_row")    # matmul lhsT for broadcast
            tot3 = pp.tile([1, 3], f32, tag="tot3")            # sum of px/py/pz
            counts_st = pp.tile([1, 500], f32, tag="counts_st")
            cent_st = pp.tile([1, 101 * 15], f32, tag="cent_st")
            # constants for gpsimd tail ops (gpsimd has no tensor_scalar)
            nconst = pp.tile([1, 1], f32, tag="nconst")
            halfc = pp.tile([1, 1], f32, tag="halfc")
            twoc = pp.tile([1, 1], f32, tag="twoc")

            nc.vector.memset(counts_st[:], 0.0)
            nc.vector.memset(cent_st[:], 0.0)
            nc.vector.memset(nconst[:], float(N))
            nc.vector.memset(halfc[:], -0.5)
            nc.vector.memset(twoc[:], 2.0)
            xap = xp[:].rearrange("c (p f) -> c p f", p=P)
            nc.sync.dma_start(out=px[:], in_=xap[0])
            nc.sync.dma_start(out=py[:], in_=xap[1])
            nc.sync.dma_start(out=pz[:], in_=xap[2])
            cb0 = pp.tile([1, 20], f32, tag="cb0")
            nc.sync.dma_start(out=cb0[:], in_=cbin[:])

            nc.vector.memset(ones_col[:], 1.0)
            nc.vector.memset(ones_row[:], 1.0)

            # pixels = x + 1e-8, vector-owned; gpsimd gets private copies so
            # its loop-body ops never need cross-engine waits (HW structs have
            # very few sync-wait slots)
            nc.vector.tensor_scalar(px[:], px[:], 1e-8, None, Alu.add)
            nc.vector.tensor_scalar(py[:], py[:], 1e-8, None, Alu.add)
            nc.vector.tensor_scalar(pz[:], pz[:], 1e-8, None, Alu.add)


            planes0 = (px, py, pz)
            # totals: [1,3] = sum of each plane
            totc = pp.tile([P, 3], f32, tag="totc")
            nc.vector.tensor_reduce(totc[:, 0:1], px[:], mybir.AxisListType.X, Alu.add)
            nc.vector.tensor_reduce(totc[:, 1:2], py[:], mybir.AxisListType.X, Alu.add)
            nc.vector.tensor_reduce(totc[:, 2:3], pz[:], mybir.AxisListType.X, Alu.add)
            tot3_ps = ps.tile([1, 3], f32, tag="tot3ps")
            nc.tensor.matmul(tot3_ps[:], ones_col[:], totc[:], start=True, stop=True)
            nc.vector.tensor_copy(tot3[:], tot3_ps[:])

            # interleaved pixel tile [p, f*3] = (x,y,z) per pixel, for the
            # one-TT-per-cluster product in phase 3
            pint = pp.tile([P, 3 * F], f32, tag="pint")
            for d in range(3):
                nc.vector.tensor_copy(
                    pint[:].rearrange("p (f d) -> p d f", d=3)[:, d], planes0[d][:]
                )

            # initial centers into trajectory + initial rep broadcast
            nc.scalar.copy(cent_st[0:1, 0:15], cb0[0:1, 0:15])

            cb0v = pp.tile([1, 20], f32, tag="cb0v")
            nc.vector.tensor_copy(cb0v[:], cb0[:])
            rep_ps0 = ps.tile([P, 20], f32, tag="repps")
            nc.tensor.matmul(rep_ps0[:], ones_row[:], cb0v[:], start=True, stop=True)
            rep = sb.tile([P, 20], f32, tag="rep")
            nc.vector.tensor_copy(rep[:], rep_ps0[:])

            for t in range(1, ITERS + 1):
                # ---------- phase 1: scores s_k = px*cx + py*cy + pz*cz + b ----------
                s5 = sb.tile([P, 5 * F], f32, tag="s5")
                u_tiles = []
                for k in range(5):
                    u = scr.tile([P, F], f32, tag=f"u{k}")
                    # u = px*cx_k + b_k (ACT free affine with AP scale/bias)
                    nc.scalar.activation(
                        u[:], px[:], Act.Identity,
                        bias=rep[:, 15 + k : 16 + k], scale=rep[:, 3 * k : 3 * k + 1],
                    )
                    u_tiles.append(u)
                for k in range(5):
                    v = scr.tile([P, F], f32, tag=f"v{k}")
                    nc.vector.scalar_tensor_tensor(
                        v[:], py[:], rep[:, 3 * k + 1 : 3 * k + 2], u_tiles[k][:],
                        Alu.mult, Alu.add,
                    )
                    nc.vector.scalar_tensor_tensor(
                        s5[:, k * F : (k + 1) * F], pz[:],
                        rep[:, 3 * k + 2 : 3 * k + 3], v[:], Alu.mult, Alu.add,
                    )

                # ---------- phase 2: m = max_k s_k (one segmented reduce) ----------
                m = sb.tile([P, F], f32, tag="m")
                nc.vector.tensor_reduce(
                    m[:], s5[:].rearrange("p (k f) -> p f k", k=5),
                    mybir.AxisListType.X, Alu.max,
                )

                # ---------- phase 3: masks (one fused TT), counts, products, sums ----------
                # acc_d: 0:4 cnt0..3 | 4:16 S k-major | 16 csum
                acc_d = sb.tile([P, 17], f32, tag="acc_d")
                junk_a = scr.tile([P, F], f32, tag="junk_a")
                maskQ = sb.tile([P, 4 * F], f32, tag="maskQ")
                nc.vector.tensor_tensor(
                    maskQ[:].rearrange("p (k f) -> p k f", k=4),
                    s5[:, 0 : 4 * F].rearrange("p (k f) -> p k f", k=4),
                    m[:].rearrange("p (o f) -> p o f", o=1).broadcast_to((P, 4, F)),
                    Alu.is_equal,
                )
                mask_tiles = [maskQ[:, k * F : (k + 1) * F] for k in range(4)]
                for k in range(4):
                    nc.scalar.activation(
                        junk_a[:], mask_tiles[k], Act.Identity,
                        accum_out=acc_d[:, k : k + 1],
                    )
                junk4 = sm.tile([P, 4], f32, tag="junk4")
                nc.scalar.activation(
                    junk4[:], acc_d[:, 0:4], Act.Identity,
                    accum_out=acc_d[:, 16:17],
                )

                # products: clusters 0 on DVE (strided one-op); 1,2,3 on GpSimd
                # (plain contiguous per-channel TTs). prod memory is d-major.
                prods = []
                for k in range(4):
                    prod3 = scr.tile([P, 3 * F], f32, tag=f"prod{k}")
                    prods.append(prod3)
                for k in (2, 3):
                    nc.gpsimd.tensor_tensor(
                        prods[k][:].rearrange("p (d f) -> p f d", f=F),
                        mask_tiles[k].rearrange("p (f o) -> p f o", o=1).broadcast_to((P, F, 3)),
                        pint[:].rearrange("p (f d) -> p f d", d=3),
                        Alu.mult,
                    )
                for k in (0, 1):
                    nc.vector.tensor_tensor(
                        prods[k][:].rearrange("p (d f) -> p f d", f=F),
                        mask_tiles[k].rearrange("p (f o) -> p f o", o=1).broadcast_to((P, F, 3)),
                        pint[:].rearrange("p (f d) -> p f d", d=3),
                        Alu.mult,
                    )
                # sums: clusters 0,1 via DVE segmented reduce; 2,3 via ACT accum
                for k in (0, 1):
                    nc.vector.tensor_reduce(
                        acc_d[:, 4 + 3 * k : 7 + 3 * k],
                        prods[k][:].rearrange("p (d f) -> p d f", d=3),
                        mybir.AxisListType.X, Alu.add,
                    )
                for k in (2, 3):
                    for d in range(3):
                        nc.scalar.activation(
                            junk_a[:], prods[k][:, d * F : (d + 1) * F], Act.Identity,
                            accum_out=acc_d[:, 4 + 3 * k + d : 5 + 3 * k + d],
                        )

                # ---------- tail: totals -> new centers ----------
                tot = ps.tile([1, 17], f32, tag="tot")
                nc.tensor.matmul(tot[:], ones_col[:], acc_d[:], start=True, stop=True)
                tots = sm.tile([1, 17], f32, tag="tots")
                nc.scalar.copy(tots[:], tot[:])

                cnts = sm.tile([1, 5], f32, tag="cnts")
                nc.gpsimd.tensor_copy(cnts[0:1, 0:4], tots[0:1, 0:4])
                nc.gpsimd.tensor_tensor(
                    cnts[0:1, 4:5], nconst[:], tots[0:1, 16:17], Alu.subtract
                )

                S15 = sm.tile([1, 15], f32, tag="S15")
                s4p = sm.tile([1, 3], f32, tag="s4p")
                nc.gpsimd.tensor_copy(S15[0:1, 0:12], tots[0:1, 4:16])
                # sum over k of S_kd: view cols 4..16 as [d, k(stride3)], reduce X
                nc.vector.tensor_reduce(
                    s4p[:], tots[0:1, 4:16].rearrange("p (k d) -> p d k", d=3),
                    mybir.AxisListType.X, Alu.add,
                )
                nc.gpsimd.tensor_tensor(S15[0:1, 12:15], tot3[:], s4p[:], Alu.subtract)

                recip = sm.tile([1, 5], f32, tag="recip")
                nc.vector.reciprocal(recip[:], cnts[:])

                cb = sm.tile([1, 20], f32, tag="cb")
                nc.gpsimd.tensor_tensor(
                    cb[0:1, 0:15].rearrange("p (k d) -> p k d", d=3),
                    S15[:].rearrange("p (k d) -> p k d", d=3),
                    recip[:].rearrange("p (k o) -> p k o", o=1).broadcast_to((1, 5, 3)),
                    Alu.mult,
                )

                sq = sm.tile([1, 15], f32, tag="sq")
                c2 = sm.tile([1, 5], f32, tag="c2")
                nc.gpsimd.tensor_tensor(sq[:], cb[0:1, 0:15], cb[0:1, 0:15], Alu.mult)
                nc.vector.tensor_reduce(
                    c2[:], sq[:].rearrange("p (k d) -> p k d", d=3),
                    mybir.AxisListType.X, Alu.add,
                )
                nc.gpsimd.tensor_tensor(
                    c2[:], c2[:], halfc[:].broadcast_to((1, 5)), Alu.mult
                )
                nc.gpsimd.tensor_tensor(
                    cb[0:1, 15:20], twoc[:].broadcast_to((1, 5)), c2[:], Alu.add
                )

                # store trajectories (ScalarE, off critical path)
                nc.scalar.copy(counts_st[0:1, 5 * (t - 1) : 5 * t], cnts[:])
                nc.scalar.copy(cent_st[0:1, 15 * t : 15 * (t + 1)], cb[0:1, 0:15])

                # broadcast for next iteration
                rep_ps = ps.tile([P, 20], f32, tag="repps")
                nc.tensor.matmul(rep_ps[:], ones_row[:], cb[:], start=True, stop=True)
                rep = sb.tile([P, 20], f32, tag="rep")
                nc.scalar.copy(rep[:], rep_ps[:])

            nc.sync.dma_start(out=outv[0:1, 0:500], in_=counts_st[:])
            nc.sync.dma_start(out=outv[0:1, 500:OUT_LEN], in_=cent_st[:])
    nc.compile()
    return nc


def _get_nc():
    if "nc" not in _CACHE:
        _CACHE["nc"] = _build_nc()
    return _CACHE["nc"]


def _host_finalize(counts_all, cent_all):
    """counts_all [B,100,5], cent_all [B,101,15] -> [B,K,K,4] per reference."""
    B = counts_all.shape[0]
    prev = cent_all[:, :-1, :]   # centers entering iter t (t=1..100)
    new = cent_all[:, 1:, :]     # new_centers at iter t
    with np.errstate(invalid="ignore"):
        ok = np.abs(prev - new) <= np.float32(ATOL) + np.float32(RTOL) * np.abs(new)
    conv_t = np.all(ok, axis=(0, 2))          # [100] global allclose per iter
    idx = np.nonzero(conv_t)[0]
    T = int(idx[0]) + 1 if len(idx) else ITERS + 1
    L = min(T, ITERS)
    centers = cent_all[:, T - 1].reshape(B, K, 3)
    percentages = counts_all[:, L - 1] / np.float32(N)
    centers = np.clip(centers, 0.0, 1.0)
    percentages = np.clip(percentages, 0.0, 1.0)
    color_info = np.concatenate([centers, percentages[..., None]], axis=2).astype(np.float32)
    color_info = np.nan_to_num(color_info, nan=0.0, posinf=1.0, neginf=0.0)
    sort_idx = np.argsort(-color_info[:, :, 3], axis=1, kind="stable")
    return color_info[sort_idx]


def _make_inputs(x, init_idx):
    B = x.shape[0]
    x = np.ascontiguousarray(np.asarray(x, dtype=np.float32))
    init_idx = np.asarray(init_idx).astype(np.int64)
    hh, ww = init_idx // 224, init_idx % 224
    in_maps = []
    for b in range(B):
        c0 = (x[b, :, hh, ww] + np.float32(1e-8)).astype(np.float32)  # [5,3]
        cb0 = np.zeros((1, 20), np.float32)
        cb0[0, :15] = c0.reshape(15)
        c2 = (c0 * c0).sum(axis=1, dtype=np.float32)
        cb0[0, 15:20] = np.float32(2.0) - np.float32(0.5) * c2
        in_maps.append({"xp": x[b].reshape(3, N), "cbin": cb0})
    return in_maps


def kernel(x, init_idx):
    from concourse.bass_utils import run_bass_kernel_spmd

    nc = _get_nc()
    in_maps = _make_inputs(x, init_idx)
    res = run_bass_kernel_spmd(nc, in_maps, list(range(8)))
    outs = [np.asarray(r["outv"]).reshape(OUT_LEN) for r in res.results]
    counts_all = np.stack([o[0:500].reshape(100, 5) for o in outs])
    cent_all = np.stack([o[500:OUT_LEN].reshape(101, 15) for o in outs])
    return _host_finalize(counts_all, cent_all)
